# revision 57
# baseline (speedup 1.0000x reference)
"""NetVLAD pooling kernel for Trainium2 (Bass/Tile), SPMD over 8 NeuronCores.

Reference computation (per sample n):
    x_hat = x / ||x||_C                      # L2 norm over channels, per position
    logits = fc_w @ x_hat + fc_b             # [K, S]
    soft = softmax_K(logits)                 # [K, S]
    a_sum = soft.sum(S)                      # [K]
    vlad = soft @ x_hat^T - a_sum[:,None] * centroids     # [K, C]
    vlad = intra_l2norm(vlad) ; flatten ; global l2norm

Kernel strategy (per core, data-parallel over N; group = 1024 positions =
8 tiles of 128):
  Per tile (PE, f16):
    mm-xT:   stationary x tile [c,128], rhs = I        -> xT PSUM   [s,128]
    mm-log:  same stationary,  rhs = fc_w^T (64 cols)  -> logits PSUM [s,64]
    mm-ssq:  stationary x^2 f16 tile, rhs = ones col   -> ssq PSUM  [s,1]
  PSUM layout per group: xT [128,1024]f32 (2 banks, 2 bufs), logits
  [128,512]f32 (1 bank, 2 bufs), ssq [128,8] (1 bank), vlad [64,132]
  (1 bank) = 8 banks.
  Per-group chain, layout B (s on partitions):
    ACT uses ONLY the natural_log_exp table set (ln/exp/copy/square) --
    zero table-set switches (sqrt is computed as exp(0.5*ln(ssq)); bacc's
    per-function greedy set choice is overridden post-finalize by
    _pin_act_tables, else Ln/Exp alternate natural_log <-> exp_and_others
    at ~1.3us per switch, every group):
      L = ln(ssq); rnorm = exp(-0.5 L)
      xt norm col <- exp(+0.5 L) written directly by ACT
    t1 = logits * rnorm (DVE 3D), t2 = t1 + biasg (DVE), E = exp(t2) (ACT)
    sume = reduce_K(E) (DVE 3D), qsc = rnorm/sume (DVE recip+mul)
    wt = E * qsc (DVE); xt cols 0:128 = x^T PSUM->SBUF f16 (ACT, small
    DVE share)
  mm2 (PE, f16): stationary = wt tile [s,64], moving = [xT|normv] (129)
    -> accum vlad[k,0:128], a_sum col 128 (= sum_s w*normv = sum soft)
  Epilogue per sample: vlad - a_sum*centroids, intra-norm, global norm
  = /8 folded (the final L2 norm is exactly sqrt(K)=8).

Emission is software-pipelined (front(g); tail(g-1); mid(g)) and the
x^T path stays on the PE: an XBAR transpose-DMA variant (xt_mode="dma")
was measured slower on HW (DMA-channel serialization) and the combined
[I|fcwT] single-matmul variant loses to the split form at group=1024
(drift-controlled interleaved A/B).
"""

import contextlib
import numpy as np

import concourse.bacc as bacc
import concourse.bass as bass
import concourse.mybir as mybir
import concourse.tile as tile

N, C, S, K = 16, 128, 16384, 64
N_CORES = 8
N_PER_CORE = N // N_CORES  # 2

F32 = mybir.dt.float32
F16 = mybir.dt.float16
AF = mybir.ActivationFunctionType
ALU = mybir.AluOpType
AX = mybir.AxisListType

TILE = 128           # positions per matmul tile

# tuning knobs (overridable via build_nc(opts=...))
DEFAULT_OPTS = dict(
    group=1024,         # positions per matmul/chain group (8 tiles)
    mm1_mode="split",   # "split": separate xT/logits matmuls (PSUM fits at
                        # group=1024); "combined": one [I|fcwT] mm per tile
    mm1_bufs=3,         # [combined] PSUM bufs (2 banks each)
    xt_mode="pe",
    xt_dve_cols=16,     # share of x^T PSUM->SBUF copy on DVE (rest ACT)
    xt_pool_cols=0,     # share on Pool (gpsimd 3D strided copy breaks
                        # walrus lowering -- keep 0)
    x2_engine="act",    # "dve" | "act" | "pool": who squares x for ssq
    x2_dve_cols=650,    # if set with x2_engine="act": split cols on DVE
    t2_engine="dve",    # "dve" | "pool": bias add
    load_groups=1,      # groups per x DMA / cast / square block
    xt_bufs=5,
    xf_bufs=4,
    ew_bufs=5,
    mm_xt_bufs=2,       # [split] PSUM bufs for x^T (2 banks each)
    mm_lg_bufs=2,       # [split] PSUM bufs for logits (1 bank each)
)


def build_nc(n_samples=N_PER_CORE, s_len=S, finalize=True, repeat=1, opts=None,
             repeat_mode="for"):
    """Build the Bass module for one core processing `n_samples` samples."""
    o = dict(DEFAULT_OPTS)
    if opts:
        o.update(opts)
    group = o["group"]
    tpg = group // TILE

    nc = bacc.Bacc("TRN2", target_bir_lowering=False, debug=False)

    x_d = nc.dram_tensor("x", [n_samples, C, s_len], F32, kind="ExternalInput")
    fcw_d = nc.dram_tensor("fc_w", [K, C], F32, kind="ExternalInput")
    fcb_d = nc.dram_tensor("fc_b", [1, K], F32, kind="ExternalInput")
    cent_d = nc.dram_tensor("centroids", [K, C], F32, kind="ExternalInput")
    out_d = nc.dram_tensor("out", [n_samples, K, C], F32, kind="ExternalOutput")

    n_groups = s_len // group

    with tile.TileContext(nc) as tc:
        with (
            tc.tile_pool(name="const", bufs=1) as const_pool,
            tc.tile_pool(name="xf", bufs=o["xf_bufs"]) as x_pool,
            tc.tile_pool(name="xh", bufs=o["xf_bufs"]) as xh_pool,
            tc.tile_pool(name="x2", bufs=2) as x2_pool,
            tc.tile_pool(name="xt", bufs=o["xt_bufs"]) as xt_pool,
            tc.tile_pool(name="ew", bufs=o["ew_bufs"]) as ew_pool,
            tc.tile_pool(name="sm", bufs=3) as sm_pool,
            tc.tile_pool(name="ep", bufs=1) as ep_pool,
        ):
            # ---------------- constants ----------------
            ones_f32 = const_pool.tile([128, 128], F32, tag="ones_f32")
            nc.vector.memset(ones_f32[:], 1.0)
            ident_f32 = const_pool.tile([128, 128], F32, tag="ident_f32")
            nc.gpsimd.affine_select(
                ident_f32[:], ones_f32[:], pattern=[[1, 128]],
                compare_op=ALU.is_equal, fill=0.0, base=0, channel_multiplier=-1,
            )
            ident_f16 = const_pool.tile([128, 128], F16, tag="ident_f16")
            nc.vector.tensor_copy(ident_f16[:], ident_f32[:])
            ones_col_f16 = const_pool.tile([128, 1], F16, tag="ones_col")
            nc.vector.memset(ones_col_f16[:], 1.0)

            fcw_sb = const_pool.tile([K, C], F32, tag="fcw")
            nc.sync.dma_start(out=fcw_sb[:], in_=fcw_d.ap())
            fcb_sb = const_pool.tile([1, K], F32, tag="fcb")
            nc.sync.dma_start(out=fcb_sb[:], in_=fcb_d.ap())
            cent_sb = const_pool.tile([K, C], F32, tag="cent")
            nc.sync.dma_start(out=cent_sb[:], in_=cent_d.ap())

            fcwT = const_pool.tile([128, K], F16, tag="fcwT")
            biasg = const_pool.tile([128, tpg * K], F16, tag="biasg")
            combined = o.get("mm1_mode", "split") == "combined"
            if combined:
                rhs_const = const_pool.tile([128, 128 + K], F16,
                                            tag="rhs_const")
                nc.vector.tensor_copy(rhs_const[:, 0:128], ident_f32[:])
            else:
                rhs_const = None
            with tc.tile_pool(name="ipsum", bufs=1, space="PSUM") as ipsum_pool:
                fcwT_psum = ipsum_pool.tile([128, K], F32, tag="init")
                nc.tensor.transpose(fcwT_psum[:], fcw_sb[:], ident_f32[0:K, 0:K])
                nc.vector.tensor_copy(fcwT[:], fcwT_psum[:])
                if combined:
                    nc.vector.tensor_copy(rhs_const[:, 128:128 + K],
                                          fcwT_psum[:])

                # biasg[s, t*K + k] = fc_b[k]
                bias_psum = ipsum_pool.tile([128, K], F32, tag="init")
                nc.tensor.matmul(
                    bias_psum[:], lhsT=ones_f32[0:1, :], rhs=fcb_sb[:],
                    start=True, stop=True, skip_group_check=True,
                )
                bg_3d = biasg[:].rearrange("p (t x) -> p t x", t=tpg)
                nc.vector.tensor_copy(
                    bg_3d, bias_psum[:].unsqueeze(1).broadcast_to((128, tpg, K)))

            if combined:
                # [x^T | logits] interleaved at 256 stride (v1-style);
                # group must be 512 so 3 bufs of 2 banks fit PSUM
                mmxt_ctx = tc.tile_pool(name="mm1", bufs=o.get("mm1_bufs", 3),
                                        space="PSUM")
                mmlg_ctx = contextlib.nullcontext()
            else:
                mmxt_ctx = (
                    tc.tile_pool(name="mmxt", bufs=o["mm_xt_bufs"],
                                 space="PSUM")
                    if o["xt_mode"] == "pe" else contextlib.nullcontext()
                )
                mmlg_ctx = tc.tile_pool(name="mmlg", bufs=o["mm_lg_bufs"],
                                        space="PSUM")
            ssq_bufs = 2 if o["xt_mode"] == "dma" else 1
            with (
                mmxt_ctx as mmxt_pool,
                mmlg_ctx as mmlg_pool,
                tc.tile_pool(name="ssqp", bufs=ssq_bufs,
                             space="PSUM") as ssq_pool,
                tc.tile_pool(name="vladp", bufs=1, space="PSUM") as vlad_pool,
            ):
                env = dict(
                    o=o, group=group, tpg=tpg, n_groups=n_groups,
                    n_samples=n_samples,
                    x_pool=x_pool, xh_pool=xh_pool, x2_pool=x2_pool,
                    mmxt_pool=(None if o["xt_mode"] == "dma" else mmxt_pool),
                    mmlg_pool=mmlg_pool,
                    ssq_pool=ssq_pool, vlad_pool=vlad_pool,
                    xt_pool=xt_pool, ew_pool=ew_pool, sm_pool=sm_pool,
                    ep_pool=ep_pool,
                    ident_f16=ident_f16, fcwT=fcwT, biasg=biasg,
                    ones_col_f16=ones_col_f16, cent_sb=cent_sb,
                    rhs_const=rhs_const, combined=combined,
                )

                if repeat > 1 and repeat_mode == "unroll":
                    for _ in range(repeat):
                        _main_body(nc, x_d.ap(), out_d.ap(), env)
                elif repeat > 1 and repeat_mode.startswith("hybrid"):
                    k = int(repeat_mode.split(":")[1])
                    assert repeat % k == 0
                    with tc.For_i(0, repeat // k, 1):
                        for _ in range(k):
                            _main_body(nc, x_d.ap(), out_d.ap(), env)
                else:
                    loop_ctx = (tc.For_i(0, repeat, 1) if repeat > 1
                                else contextlib.nullcontext())
                    with loop_ctx:
                        _main_body(nc, x_d.ap(), out_d.ap(), env)

    if finalize:
        nc.finalize()
        _pin_act_tables(nc)
    return nc


def _pin_act_tables(nc):
    """Replace the per-function ACT table-set loads with one load of a set
    covering every activation function we use.

    bacc's insert_act_table_loads picks the FIRST act_info.json set
    containing each function, so an Ln/Exp mix alternates between
    `natural_log` and `exp_and_others` — a ~1.3us table DMA per switch,
    every group. All our functions (Ln, Exp, Copy, Square, Identity) live
    together in `natural_log_exp_and_others`, so one load up front
    suffices. Runs post-finalize: rewrites this module's own instructions
    only.
    """
    from concourse.hw_specs import get_activation_tables

    used = set()
    for b in nc.main_func.blocks:
        for inst in b.instructions:
            if isinstance(inst, mybir.InstActivation):
                used.add(inst.func)
    if not used:
        return
    tables = list(get_activation_tables(nc.m.arch).items())
    target = None
    for idx, (name, funcs) in enumerate(tables):
        if used <= funcs:
            target = idx
            break
    if target is None:  # no single covering set: leave the default placement
        return

    first_load = None
    for b in nc.main_func.blocks:
        keep = []
        for inst in b.instructions:
            if isinstance(inst, mybir.InstLoadActFuncSet):
                if first_load is None:
                    inst.act_func_set_id = target
                    first_load = inst
                continue  # drop all loads (the kept one is re-inserted below)
            keep.append(inst)
        b.instructions[:] = keep
    if first_load is not None:
        nc.main_func.blocks[0].instructions.insert(0, first_load)


def _main_body(nc, x_ap, out_ap, env):
    o = env["o"]
    group, tpg = env["group"], env["tpg"]
    n_samples, n_groups = env["n_samples"], env["n_groups"]
    x_pool = env["x_pool"]; xh_pool = env["xh_pool"]; x2_pool = env["x2_pool"]
    mmxt_pool = env["mmxt_pool"]; mmlg_pool = env["mmlg_pool"]
    ssq_pool = env["ssq_pool"]; vlad_pool = env["vlad_pool"]
    xt_pool = env["xt_pool"]; ew_pool = env["ew_pool"]; sm_pool = env["sm_pool"]
    ep_pool = env["ep_pool"]
    ident_f16 = env["ident_f16"]; fcwT = env["fcwT"]; biasg = env["biasg"]
    ones_col_f16 = env["ones_col_f16"]; cent_sb = env["cent_sb"]
    xt_dve = o["xt_dve_cols"]
    combined = env["combined"]; rhs_const = env["rhs_const"]
    lgrp = o.get("load_groups", 2)     # groups per x DMA/cast/x2 op
    lsz = lgrp * group

    def front_load(n, p):
        """Load + cast + square for a block of `lgrp` groups."""
        xf = x_pool.tile([128, lsz], F32)
        nc.sync.dma_start(out=xf[:], in_=x_ap[n][:, p * lsz:(p + 1) * lsz])
        xh = xh_pool.tile([128, lsz], F16)
        nc.gpsimd.tensor_copy(xh[:], xf[:])
        x2 = x2_pool.tile([128, lsz], F16)
        if o["x2_engine"] == "dve":
            nc.vector.tensor_mul(x2[:], xh[:], xh[:])
        elif o["x2_engine"] == "act":
            xd = o["x2_dve_cols"]
            if xd:
                nc.vector.tensor_mul(x2[:, 0:xd], xh[:, 0:xd], xh[:, 0:xd])
                nc.scalar.activation(x2[:, xd:], xh[:, xd:], func=AF.Square)
            else:
                nc.scalar.activation(x2[:], xh[:], func=AF.Square)
        else:
            nc.gpsimd.tensor_mul(x2[:], xh[:], xh[:])
        return {"xh": xh, "x2": x2}

    def front(n, g, blk):
        """Matmuls for group g (slices of the current load block)."""
        st = {}
        off = (g % lgrp) * group
        xh = blk["xh"][:, off:off + group]
        x2 = blk["x2"][:, off:off + group]

        if combined:
            mm1p = mmxt_pool.tile([128, tpg * 256], F32, tag="mm1p")
            st["mm1p"] = mm1p
        else:
            xTp = mmxt_pool.tile([128, group], F32, tag="xTp")
            st["xTp"] = xTp
            lgp = mmlg_pool.tile([128, tpg * K], F32)
            st["lgp"] = lgp
        ssqp = ssq_pool.tile([128, tpg], F32)
        for t in range(tpg):
            lhs = xh[:, t * TILE:(t + 1) * TILE]
            if combined:
                nc.tensor.matmul(
                    mm1p[:, t * 256: t * 256 + 128 + K], lhsT=lhs,
                    rhs=rhs_const[:],
                    start=True, stop=True, skip_group_check=True,
                )
            else:
                nc.tensor.matmul(
                    st["xTp"][:, t * TILE:(t + 1) * TILE], lhsT=lhs,
                    rhs=ident_f16[:],
                    start=True, stop=True, skip_group_check=True,
                )
                nc.tensor.matmul(
                    lgp[:, t * K:(t + 1) * K], lhsT=lhs, rhs=fcwT[:],
                    start=True, stop=True, skip_group_check=True,
                )
            nc.tensor.matmul(
                ssqp[:, t:t + 1],
                lhsT=x2[:, t * TILE:(t + 1) * TILE],
                rhs=ones_col_f16[:],
                start=True, stop=True, skip_group_check=True,
            )
        st["ssqp"] = ssqp
        return st

    def mid(n, g, st):
        """Norms + x^T copy + t1/t2 + E for group g."""
        ssqp = st["ssqp"]
        xt = xt_pool.tile([128, tpg * 132], F16)
        xt_3d = xt[:].rearrange("p (t x) -> p t x", t=tpg)
        st["xt"] = xt
        if combined:
            mm1_3d = st["mm1p"][:].rearrange("p (t x) -> p t x", t=tpg)
            lg_3d = mm1_3d[:, :, 128:128 + K]
            xT_3d = mm1_3d[:, :, 0:128]
        else:
            lg_3d = st["lgp"][:].rearrange("p (t x) -> p t x", t=tpg)
            xT_3d = st["xTp"][:].rearrange("p (t x) -> p t x", t=tpg)

        # norms from ssq via ln/exp (single ACT table set):
        #   rnorm = exp(-0.5 ln ssq) = 1/||x||; the norm column of xt gets
        #   normv = exp(+0.5 ln ssq) written directly by ACT
        lnv = sm_pool.tile([128, tpg], F32, tag="lnv")
        nc.scalar.activation(lnv[:], ssqp[:], func=AF.Ln)
        rnorm = sm_pool.tile([128, tpg], F32, tag="rnorm")
        nc.scalar.activation(rnorm[:], lnv[:], func=AF.Exp, scale=-0.5)
        nc.scalar.activation(xt_3d[:, :, 128:129], lnv[:].unsqueeze(-1),
                             func=AF.Exp, scale=0.5)
        rnorm_b = rnorm[:].unsqueeze(-1).broadcast_to((128, tpg, K))
        st["rnorm"] = rnorm

        # x^T PSUM->SBUF f16 copy
        xt_pool_cols = o.get("xt_pool_cols", 0)
        lo, hi = xt_dve, 128 - xt_pool_cols
        if lo > 0:
            nc.vector.tensor_copy(xt_3d[:, :, 0:lo], xT_3d[:, :, 0:lo])
        if hi > lo:
            nc.scalar.copy(xt_3d[:, :, lo:hi], xT_3d[:, :, lo:hi])
        if xt_pool_cols > 0:
            nc.gpsimd.tensor_copy(xt_3d[:, :, hi:128], xT_3d[:, :, hi:128])

        # t2 = raw*rnorm + bias ; E = exp(t2)
        t1 = ew_pool.tile([128, tpg * K], F16, tag="t1")
        t1_3d = t1[:].rearrange("p (t x) -> p t x", t=tpg)
        nc.vector.tensor_mul(t1_3d, lg_3d, rnorm_b)
        t2 = ew_pool.tile([128, tpg * K], F16, tag="t2")
        if o["t2_engine"] == "pool":
            nc.gpsimd.tensor_add(t2[:], t1[:], biasg[:])
        else:
            nc.vector.tensor_add(t2[:], t1[:], biasg[:])
        E = ew_pool.tile([128, tpg * K], F16, tag="E")
        nc.scalar.activation(E[:], t2[:], func=AF.Exp)
        st["E"] = E

    def tail(n, g, st, vlad_psum):
        """Softmax normalization + mm2 for group g."""
        E = st["E"]; xt = st["xt"]; rnorm = st["rnorm"]
        E_3d = E[:].rearrange("p (t x) -> p t x", t=tpg)

        sume = sm_pool.tile([128, tpg], F32, tag="sume")
        nc.vector.tensor_reduce(sume[:], E_3d, axis=AX.X, op=ALU.add)
        # qsc = rnorm / sumexp (walrus cannot lower a DVE divide, so
        # reciprocal + multiply; the multiply can ride on Pool)
        rsum = sm_pool.tile([128, tpg], F32, tag="rsum")
        nc.vector.reciprocal(rsum[:], sume[:])
        qsc = sm_pool.tile([128, tpg], F32, tag="qsc")
        if o.get("qsc_engine", "dve") == "pool":
            nc.gpsimd.tensor_mul(qsc[:], rsum[:], rnorm[:])
        else:
            nc.vector.tensor_mul(qsc[:], rsum[:], rnorm[:])
        wt = ew_pool.tile([128, tpg * K], F16, tag="wt")
        wt_3d = wt[:].rearrange("p (t x) -> p t x", t=tpg)
        q_b = qsc[:].unsqueeze(-1).broadcast_to((128, tpg, K))
        nc.vector.tensor_mul(wt_3d, E_3d, q_b)

        for t in range(tpg):
            first = (g == 0 and t == 0)
            last = (g == n_groups - 1 and t == tpg - 1)
            nc.tensor.matmul(
                vlad_psum[:, 0:129],
                lhsT=wt[:, t * K:(t + 1) * K],
                rhs=xt[:, t * 132: t * 132 + 129],
                start=first, stop=last, skip_group_check=True,
            )

    pipeline = o.get("pipeline", True)
    for n in range(n_samples):
        vlad_psum = vlad_pool.tile([K, 132], F32)
        prev = None
        blk = None
        for g in range(n_groups):
            if g % lgrp == 0:
                blk = front_load(n, g // lgrp)
            st = front(n, g, blk)
            if pipeline:
                if prev is not None:
                    tail(n, g - 1, prev, vlad_psum)
                mid(n, g, st)
                prev = st
            else:
                mid(n, g, st)
                tail(n, g, st, vlad_psum)
        if prev is not None:
            tail(n, n_groups - 1, prev, vlad_psum)

        # -------- epilogue for sample n --------
        acs = ep_pool.tile([K, C], F32, tag="acs")
        nc.vector.tensor_scalar_mul(acs[:], cent_sb[:], vlad_psum[:, 128:129])
        v = ep_pool.tile([K, C], F32, tag="v")
        nc.vector.tensor_sub(v[:], vlad_psum[:, 0:128], acs[:])
        v2 = ep_pool.tile([K, C], F32, tag="v2")
        nc.vector.tensor_mul(v2[:], v[:], v[:])
        ssqv = sm_pool.tile([K, 1], F32, tag="ssqv")
        nc.vector.tensor_reduce(ssqv[:], v2[:], axis=AX.X, op=ALU.add)
        lnav = sm_pool.tile([K, 1], F32, tag="lnav")
        nc.scalar.activation(lnav[:], ssqv[:], func=AF.Ln)
        rnv = sm_pool.tile([K, 1], F32, tag="rnv")
        nc.scalar.activation(rnv[:], lnav[:], func=AF.Exp, scale=-0.5)
        o_t = ep_pool.tile([K, C], F32, tag="o")
        # v * (1/sqrt(ssqv)) * 0.125  (global L2 norm is exactly sqrt(K)=8)
        nc.vector.tensor_scalar(o_t[:], v[:], rnv[:], 0.125,
                                op0=ALU.mult, op1=ALU.mult)
        nc.sync.dma_start(out=out_ap[n], in_=o_t[:])


def kernel(x, fc_w, fc_b, centroids):
    """Full-input entry point: shards over 8 cores, returns [N, K*C] float32."""
    from concourse.bass_utils import run_bass_kernel_spmd

    x = np.ascontiguousarray(np.asarray(x, dtype=np.float32))
    fc_w = np.ascontiguousarray(np.asarray(fc_w, dtype=np.float32))
    fc_b = np.ascontiguousarray(np.asarray(fc_b, dtype=np.float32)).reshape(1, K)
    centroids = np.ascontiguousarray(np.asarray(centroids, dtype=np.float32))

    nc = build_nc(N_PER_CORE, S)
    core_ids = list(range(N_CORES))
    in_maps = []
    for i in core_ids:
        shard = x[i * N_PER_CORE:(i + 1) * N_PER_CORE]
        in_maps.append({
            "x": shard,
            "fc_w": fc_w,
            "fc_b": fc_b,
            "centroids": centroids,
        })
    # Retry transient device failures (a crashed tenant can leave the cores
    # "unrecoverable" for a minute or two; they come back on their own).
    last_exc = None
    for attempt in range(4):
        try:
            res = run_bass_kernel_spmd(nc, in_maps, core_ids)
            break
        except Exception as e:  # noqa: BLE001
            last_exc = e
            if attempt == 3:
                raise
            import time as _time
            _time.sleep(45)
    outs = [res.results[i]["out"].reshape(N_PER_CORE, K * C) for i in range(N_CORES)]
    return np.concatenate(outs, axis=0)


# revision 59
# speedup vs baseline: 1.1056x; 1.1056x over previous
"""NetVLAD pooling kernel for Trainium2 (Bass/Tile), SPMD over 8 NeuronCores.

Reference computation (per sample n):
    x_hat = x / ||x||_C                      # L2 norm over channels, per position
    logits = fc_w @ x_hat + fc_b             # [K, S]
    soft = softmax_K(logits)                 # [K, S]
    a_sum = soft.sum(S)                      # [K]
    vlad = soft @ x_hat^T - a_sum[:,None] * centroids     # [K, C]
    vlad = intra_l2norm(vlad) ; flatten ; global l2norm

Kernel strategy (per core, data-parallel over N; group = 1024 positions =
8 tiles of 128):
  Per tile (PE, f16):
    mm-xT:   stationary x tile [c,128], rhs = I        -> xT PSUM   [s,128]
    mm-log:  same stationary,  rhs = fc_w^T (64 cols)  -> logits PSUM [s,64]
    mm-ssq:  stationary x^2 f16 tile, rhs = ones col   -> ssq PSUM  [s,1]
  PSUM layout per group: xT [128,1024]f32 (2 banks, 2 bufs), logits
  [128,512]f32 (1 bank, 2 bufs), ssq [128,8] (1 bank), vlad [64,132]
  (1 bank) = 8 banks.
  Per-group chain, layout B (s on partitions):
    ACT uses ONLY the natural_log_exp table set (ln/exp/copy/square) --
    zero table-set switches (sqrt is computed as exp(0.5*ln(ssq)); bacc's
    per-function greedy set choice is overridden post-finalize by
    _pin_act_tables, else Ln/Exp alternate natural_log <-> exp_and_others
    at ~1.3us per switch, every group):
      L = ln(ssq); rnorm = exp(-0.5 L)
      xt norm col <- exp(+0.5 L) written directly by ACT
    t1 = logits * rnorm (DVE 3D), t2 = t1 + biasg (DVE), E = exp(t2) (ACT)
    sume = reduce_K(E) (DVE 3D), qsc = rnorm/sume (DVE recip+mul)
    wt = E * qsc (DVE); xt cols 0:128 = x^T PSUM->SBUF f16 (ACT, small
    DVE share)
  mm2 (PE, f16): stationary = wt tile [s,64], moving = [xT|normv] (129)
    -> accum vlad[k,0:128], a_sum col 128 (= sum_s w*normv = sum soft)
  Epilogue per sample: vlad - a_sum*centroids, intra-norm, global norm
  = /8 folded (the final L2 norm is exactly sqrt(K)=8).

Emission is software-pipelined (front(g); tail(g-1); mid(g)) and the
x^T path stays on the PE: an XBAR transpose-DMA variant (xt_mode="dma")
was measured slower on HW (DMA-channel serialization) and the combined
[I|fcwT] single-matmul variant loses to the split form at group=1024
(drift-controlled interleaved A/B).
"""

import contextlib
import numpy as np

import concourse.bacc as bacc
import concourse.bass as bass
import concourse.mybir as mybir
import concourse.tile as tile

N, C, S, K = 16, 128, 16384, 64
N_CORES = 8
N_PER_CORE = N // N_CORES  # 2

F32 = mybir.dt.float32
F16 = mybir.dt.float16
AF = mybir.ActivationFunctionType
ALU = mybir.AluOpType
AX = mybir.AxisListType

TILE = 128           # positions per matmul tile

# tuning knobs (overridable via build_nc(opts=...))
DEFAULT_OPTS = dict(
    group=1024,         # positions per matmul/chain group (8 tiles)
    mm1_mode="split",   # "split": separate xT/logits matmuls (PSUM fits at
                        # group=1024); "combined": one [I|fcwT] mm per tile
    mm1_bufs=3,         # [combined] PSUM bufs (2 banks each)
    xt_mode="pe",
    xt_dve_cols=24,     # share of x^T PSUM->SBUF copy on DVE (rest ACT)
    xt_pool_cols=0,     # share on Pool (gpsimd 3D strided copy breaks
                        # walrus lowering -- keep 0)
    x2_engine="act",    # "dve" | "act" | "pool": who squares x for ssq
    x2_dve_cols=650,    # if set with x2_engine="act": split cols on DVE
    t2_engine="dve",    # "dve" | "pool": bias add
    load_groups=1,      # groups per x DMA / cast / square block
    xt_bufs=5,
    xf_bufs=4,
    ew_bufs=5,
    mm_xt_bufs=2,       # [split] PSUM bufs for x^T (2 banks each)
    mm_lg_bufs=2,       # [split] PSUM bufs for logits (1 bank each)
)


def build_nc(n_samples=N_PER_CORE, s_len=S, finalize=True, repeat=1, opts=None,
             repeat_mode="for"):
    """Build the Bass module for one core processing `n_samples` samples."""
    o = dict(DEFAULT_OPTS)
    if opts:
        o.update(opts)
    group = o["group"]
    tpg = group // TILE

    nc = bacc.Bacc("TRN2", target_bir_lowering=False, debug=False)

    x_d = nc.dram_tensor("x", [n_samples, C, s_len], F32, kind="ExternalInput")
    fcw_d = nc.dram_tensor("fc_w", [K, C], F32, kind="ExternalInput")
    fcb_d = nc.dram_tensor("fc_b", [1, K], F32, kind="ExternalInput")
    cent_d = nc.dram_tensor("centroids", [K, C], F32, kind="ExternalInput")
    out_d = nc.dram_tensor("out", [n_samples, K, C], F32, kind="ExternalOutput")

    n_groups = s_len // group

    with tile.TileContext(nc) as tc:
        with (
            tc.tile_pool(name="const", bufs=1) as const_pool,
            tc.tile_pool(name="xf", bufs=o["xf_bufs"]) as x_pool,
            tc.tile_pool(name="xh", bufs=o["xf_bufs"]) as xh_pool,
            tc.tile_pool(name="x2", bufs=2) as x2_pool,
            tc.tile_pool(name="xt", bufs=o["xt_bufs"]) as xt_pool,
            tc.tile_pool(name="ew", bufs=o["ew_bufs"]) as ew_pool,
            tc.tile_pool(name="sm", bufs=o.get("sm_bufs", 3)) as sm_pool,
            tc.tile_pool(name="ep", bufs=1) as ep_pool,
        ):
            # ---------------- constants ----------------
            ones_f32 = const_pool.tile([128, 128], F32, tag="ones_f32")
            nc.vector.memset(ones_f32[:], 1.0)
            ident_f32 = const_pool.tile([128, 128], F32, tag="ident_f32")
            nc.gpsimd.affine_select(
                ident_f32[:], ones_f32[:], pattern=[[1, 128]],
                compare_op=ALU.is_equal, fill=0.0, base=0, channel_multiplier=-1,
            )
            ident_f16 = const_pool.tile([128, 128], F16, tag="ident_f16")
            nc.vector.tensor_copy(ident_f16[:], ident_f32[:])
            ones_col_f16 = const_pool.tile([128, 1], F16, tag="ones_col")
            nc.vector.memset(ones_col_f16[:], 1.0)

            fcw_sb = const_pool.tile([K, C], F32, tag="fcw")
            nc.sync.dma_start(out=fcw_sb[:], in_=fcw_d.ap())
            fcb_sb = const_pool.tile([1, K], F32, tag="fcb")
            nc.sync.dma_start(out=fcb_sb[:], in_=fcb_d.ap())
            cent_sb = const_pool.tile([K, C], F32, tag="cent")
            nc.sync.dma_start(out=cent_sb[:], in_=cent_d.ap())

            fcwT = const_pool.tile([128, K], F16, tag="fcwT")
            biasg = const_pool.tile([128, tpg * K], F16, tag="biasg")
            combined = o.get("mm1_mode", "split") == "combined"
            if combined:
                rhs_const = const_pool.tile([128, 128 + K], F16,
                                            tag="rhs_const")
                nc.vector.tensor_copy(rhs_const[:, 0:128], ident_f32[:])
            else:
                rhs_const = None
            with tc.tile_pool(name="ipsum", bufs=1, space="PSUM") as ipsum_pool:
                fcwT_psum = ipsum_pool.tile([128, K], F32, tag="init")
                nc.tensor.transpose(fcwT_psum[:], fcw_sb[:], ident_f32[0:K, 0:K])
                nc.vector.tensor_copy(fcwT[:], fcwT_psum[:])
                if combined:
                    nc.vector.tensor_copy(rhs_const[:, 128:128 + K],
                                          fcwT_psum[:])

                # biasg[s, t*K + k] = fc_b[k]
                bias_psum = ipsum_pool.tile([128, K], F32, tag="init")
                nc.tensor.matmul(
                    bias_psum[:], lhsT=ones_f32[0:1, :], rhs=fcb_sb[:],
                    start=True, stop=True, skip_group_check=True,
                )
                bg_3d = biasg[:].rearrange("p (t x) -> p t x", t=tpg)
                nc.vector.tensor_copy(
                    bg_3d, bias_psum[:].unsqueeze(1).broadcast_to((128, tpg, K)))

            if combined:
                # [x^T | logits] interleaved at 256 stride (v1-style);
                # group must be 512 so 3 bufs of 2 banks fit PSUM
                mmxt_ctx = tc.tile_pool(name="mm1", bufs=o.get("mm1_bufs", 3),
                                        space="PSUM")
                mmlg_ctx = contextlib.nullcontext()
            else:
                mmxt_ctx = (
                    tc.tile_pool(name="mmxt", bufs=o["mm_xt_bufs"],
                                 space="PSUM")
                    if o["xt_mode"] == "pe" else contextlib.nullcontext()
                )
                mmlg_ctx = tc.tile_pool(name="mmlg", bufs=o["mm_lg_bufs"],
                                        space="PSUM")
            ssq_bufs = 2 if o["xt_mode"] == "dma" else 1
            with (
                mmxt_ctx as mmxt_pool,
                mmlg_ctx as mmlg_pool,
                tc.tile_pool(name="ssqp", bufs=ssq_bufs,
                             space="PSUM") as ssq_pool,
                tc.tile_pool(name="vladp", bufs=1, space="PSUM") as vlad_pool,
            ):
                env = dict(
                    o=o, group=group, tpg=tpg, n_groups=n_groups,
                    n_samples=n_samples,
                    x_pool=x_pool, xh_pool=xh_pool, x2_pool=x2_pool,
                    mmxt_pool=(None if o["xt_mode"] == "dma" else mmxt_pool),
                    mmlg_pool=mmlg_pool,
                    ssq_pool=ssq_pool, vlad_pool=vlad_pool,
                    xt_pool=xt_pool, ew_pool=ew_pool, sm_pool=sm_pool,
                    ep_pool=ep_pool,
                    ident_f16=ident_f16, fcwT=fcwT, biasg=biasg,
                    ones_col_f16=ones_col_f16, cent_sb=cent_sb,
                    rhs_const=rhs_const, combined=combined,
                )

                if repeat > 1 and repeat_mode == "unroll":
                    for _ in range(repeat):
                        _main_body(nc, x_d.ap(), out_d.ap(), env)
                elif repeat > 1 and repeat_mode.startswith("hybrid"):
                    k = int(repeat_mode.split(":")[1])
                    assert repeat % k == 0
                    with tc.For_i(0, repeat // k, 1):
                        for _ in range(k):
                            _main_body(nc, x_d.ap(), out_d.ap(), env)
                else:
                    loop_ctx = (tc.For_i(0, repeat, 1) if repeat > 1
                                else contextlib.nullcontext())
                    with loop_ctx:
                        _main_body(nc, x_d.ap(), out_d.ap(), env)

    if finalize:
        nc.finalize()
        _pin_act_tables(nc)
    return nc


def _pin_act_tables(nc):
    """Replace the per-function ACT table-set loads with one load of a set
    covering every activation function we use.

    bacc's insert_act_table_loads picks the FIRST act_info.json set
    containing each function, so an Ln/Exp mix alternates between
    `natural_log` and `exp_and_others` — a ~1.3us table DMA per switch,
    every group. All our functions (Ln, Exp, Copy, Square, Identity) live
    together in `natural_log_exp_and_others`, so one load up front
    suffices. Runs post-finalize: rewrites this module's own instructions
    only.
    """
    from concourse.hw_specs import get_activation_tables

    used = set()
    for b in nc.main_func.blocks:
        for inst in b.instructions:
            if isinstance(inst, mybir.InstActivation):
                used.add(inst.func)
    if not used:
        return
    tables = list(get_activation_tables(nc.m.arch).items())
    target = None
    for idx, (name, funcs) in enumerate(tables):
        if used <= funcs:
            target = idx
            break
    if target is None:  # no single covering set: leave the default placement
        return

    first_load = None
    for b in nc.main_func.blocks:
        keep = []
        for inst in b.instructions:
            if isinstance(inst, mybir.InstLoadActFuncSet):
                if first_load is None:
                    inst.act_func_set_id = target
                    first_load = inst
                continue  # drop all loads (the kept one is re-inserted below)
            keep.append(inst)
        b.instructions[:] = keep
    if first_load is not None:
        nc.main_func.blocks[0].instructions.insert(0, first_load)


def _main_body(nc, x_ap, out_ap, env):
    o = env["o"]
    group, tpg = env["group"], env["tpg"]
    n_samples, n_groups = env["n_samples"], env["n_groups"]
    x_pool = env["x_pool"]; xh_pool = env["xh_pool"]; x2_pool = env["x2_pool"]
    mmxt_pool = env["mmxt_pool"]; mmlg_pool = env["mmlg_pool"]
    ssq_pool = env["ssq_pool"]; vlad_pool = env["vlad_pool"]
    xt_pool = env["xt_pool"]; ew_pool = env["ew_pool"]; sm_pool = env["sm_pool"]
    ep_pool = env["ep_pool"]
    ident_f16 = env["ident_f16"]; fcwT = env["fcwT"]; biasg = env["biasg"]
    ones_col_f16 = env["ones_col_f16"]; cent_sb = env["cent_sb"]
    xt_dve = o["xt_dve_cols"]
    combined = env["combined"]; rhs_const = env["rhs_const"]
    lgrp = o.get("load_groups", 2)     # groups per x DMA/cast/x2 op
    lsz = lgrp * group

    def front_load(n, p):
        """Load + cast + square for a block of `lgrp` groups."""
        xf = x_pool.tile([128, lsz], F32)
        nc.sync.dma_start(out=xf[:], in_=x_ap[n][:, p * lsz:(p + 1) * lsz])
        xh = xh_pool.tile([128, lsz], F16)
        nc.gpsimd.tensor_copy(xh[:], xf[:])
        x2 = x2_pool.tile([128, lsz], F16)
        if o["x2_engine"] == "dve":
            nc.vector.tensor_mul(x2[:], xh[:], xh[:])
        elif o["x2_engine"] == "act":
            xd = o["x2_dve_cols"]
            if xd:
                nc.vector.tensor_mul(x2[:, 0:xd], xh[:, 0:xd], xh[:, 0:xd])
                nc.scalar.activation(x2[:, xd:], xh[:, xd:], func=AF.Square)
            else:
                nc.scalar.activation(x2[:], xh[:], func=AF.Square)
        else:
            nc.gpsimd.tensor_mul(x2[:], xh[:], xh[:])
        return {"xh": xh, "x2": x2}

    def front(n, g, blk):
        """Matmuls for group g (slices of the current load block)."""
        st = {}
        off = (g % lgrp) * group
        xh = blk["xh"][:, off:off + group]
        x2 = blk["x2"][:, off:off + group]

        if combined:
            mm1p = mmxt_pool.tile([128, tpg * 256], F32, tag="mm1p")
            st["mm1p"] = mm1p
        else:
            xTp = mmxt_pool.tile([128, group], F32, tag="xTp")
            st["xTp"] = xTp
            lgp = mmlg_pool.tile([128, tpg * K], F32)
            st["lgp"] = lgp
        ssqp = ssq_pool.tile([128, tpg], F32)
        for t in range(tpg):
            lhs = xh[:, t * TILE:(t + 1) * TILE]
            if combined:
                nc.tensor.matmul(
                    mm1p[:, t * 256: t * 256 + 128 + K], lhsT=lhs,
                    rhs=rhs_const[:],
                    start=True, stop=True, skip_group_check=True,
                )
            else:
                nc.tensor.matmul(
                    st["xTp"][:, t * TILE:(t + 1) * TILE], lhsT=lhs,
                    rhs=ident_f16[:],
                    start=True, stop=True, skip_group_check=True,
                )
                nc.tensor.matmul(
                    lgp[:, t * K:(t + 1) * K], lhsT=lhs, rhs=fcwT[:],
                    start=True, stop=True, skip_group_check=True,
                )
            nc.tensor.matmul(
                ssqp[:, t:t + 1],
                lhsT=x2[:, t * TILE:(t + 1) * TILE],
                rhs=ones_col_f16[:],
                start=True, stop=True, skip_group_check=True,
            )
        st["ssqp"] = ssqp
        return st

    def mid(n, g, st):
        """Norms + x^T copy + t1/t2 + E for group g."""
        ssqp = st["ssqp"]
        xt = xt_pool.tile([128, tpg * 132], F16)
        xt_3d = xt[:].rearrange("p (t x) -> p t x", t=tpg)
        st["xt"] = xt
        if combined:
            mm1_3d = st["mm1p"][:].rearrange("p (t x) -> p t x", t=tpg)
            lg_3d = mm1_3d[:, :, 128:128 + K]
            xT_3d = mm1_3d[:, :, 0:128]
        else:
            lg_3d = st["lgp"][:].rearrange("p (t x) -> p t x", t=tpg)
            xT_3d = st["xTp"][:].rearrange("p (t x) -> p t x", t=tpg)

        # norms from ssq via ln/exp (single ACT table set):
        #   rnorm = exp(-0.5 ln ssq) = 1/||x||; the norm column of xt gets
        #   normv = exp(+0.5 ln ssq) written directly by ACT
        lnv = sm_pool.tile([128, tpg], F32, tag="lnv")
        nc.scalar.activation(lnv[:], ssqp[:], func=AF.Ln)
        rnorm = sm_pool.tile([128, tpg], F32, tag="rnorm")
        nc.scalar.activation(rnorm[:], lnv[:], func=AF.Exp, scale=-0.5)
        nc.scalar.activation(xt_3d[:, :, 128:129], lnv[:].unsqueeze(-1),
                             func=AF.Exp, scale=0.5)
        rnorm_b = rnorm[:].unsqueeze(-1).broadcast_to((128, tpg, K))
        st["rnorm"] = rnorm

        # x^T PSUM->SBUF f16 copy
        xt_pool_cols = o.get("xt_pool_cols", 0)
        lo, hi = xt_dve, 128 - xt_pool_cols
        if lo > 0:
            nc.vector.tensor_copy(xt_3d[:, :, 0:lo], xT_3d[:, :, 0:lo])
        if hi > lo:
            nc.scalar.copy(xt_3d[:, :, lo:hi], xT_3d[:, :, lo:hi])
        if xt_pool_cols > 0:
            nc.gpsimd.tensor_copy(xt_3d[:, :, hi:128], xT_3d[:, :, hi:128])

        # t2 = raw*rnorm + bias ; E = exp(t2)
        t1 = ew_pool.tile([128, tpg * K], F16, tag="t1")
        t1_3d = t1[:].rearrange("p (t x) -> p t x", t=tpg)
        nc.vector.tensor_mul(t1_3d, lg_3d, rnorm_b)
        t2 = ew_pool.tile([128, tpg * K], F16, tag="t2")
        if o["t2_engine"] == "pool":
            nc.gpsimd.tensor_add(t2[:], t1[:], biasg[:])
        else:
            nc.vector.tensor_add(t2[:], t1[:], biasg[:])
        E = ew_pool.tile([128, tpg * K], F16, tag="E")
        nc.scalar.activation(E[:], t2[:], func=AF.Exp)
        st["E"] = E

    def tail(n, g, st, vlad_psum):
        """Softmax normalization + mm2 for group g."""
        E = st["E"]; xt = st["xt"]; rnorm = st["rnorm"]
        E_3d = E[:].rearrange("p (t x) -> p t x", t=tpg)

        sume = sm_pool.tile([128, tpg], F32, tag="sume")
        nc.vector.tensor_reduce(sume[:], E_3d, axis=AX.X, op=ALU.add)
        # qsc = rnorm / sumexp (walrus cannot lower a DVE divide, so
        # reciprocal + multiply; the multiply can ride on Pool)
        rsum = sm_pool.tile([128, tpg], F32, tag="rsum")
        nc.vector.reciprocal(rsum[:], sume[:])
        qsc = sm_pool.tile([128, tpg], F32, tag="qsc")
        if o.get("qsc_engine", "dve") == "pool":
            nc.gpsimd.tensor_mul(qsc[:], rsum[:], rnorm[:])
        else:
            nc.vector.tensor_mul(qsc[:], rsum[:], rnorm[:])
        wt = ew_pool.tile([128, tpg * K], F16, tag="wt")
        wt_3d = wt[:].rearrange("p (t x) -> p t x", t=tpg)
        q_b = qsc[:].unsqueeze(-1).broadcast_to((128, tpg, K))
        nc.vector.tensor_mul(wt_3d, E_3d, q_b)

        for t in range(tpg):
            first = (g == 0 and t == 0)
            last = (g == n_groups - 1 and t == tpg - 1)
            nc.tensor.matmul(
                vlad_psum[:, 0:129],
                lhsT=wt[:, t * K:(t + 1) * K],
                rhs=xt[:, t * 132: t * 132 + 129],
                start=first, stop=last, skip_group_check=True,
            )

    pipeline = o.get("pipeline", True)
    for n in range(n_samples):
        vlad_psum = vlad_pool.tile([K, 132], F32)
        prev = None
        blk = None
        for g in range(n_groups):
            if g % lgrp == 0:
                blk = front_load(n, g // lgrp)
            st = front(n, g, blk)
            if pipeline:
                if prev is not None:
                    tail(n, g - 1, prev, vlad_psum)
                mid(n, g, st)
                prev = st
            else:
                mid(n, g, st)
                tail(n, g, st, vlad_psum)
        if prev is not None:
            tail(n, n_groups - 1, prev, vlad_psum)

        # -------- epilogue for sample n --------
        acs = ep_pool.tile([K, C], F32, tag="acs")
        nc.vector.tensor_scalar_mul(acs[:], cent_sb[:], vlad_psum[:, 128:129])
        v = ep_pool.tile([K, C], F32, tag="v")
        nc.vector.tensor_sub(v[:], vlad_psum[:, 0:128], acs[:])
        v2 = ep_pool.tile([K, C], F32, tag="v2")
        nc.vector.tensor_mul(v2[:], v[:], v[:])
        ssqv = sm_pool.tile([K, 1], F32, tag="ssqv")
        nc.vector.tensor_reduce(ssqv[:], v2[:], axis=AX.X, op=ALU.add)
        lnav = sm_pool.tile([K, 1], F32, tag="lnav")
        nc.scalar.activation(lnav[:], ssqv[:], func=AF.Ln)
        rnv = sm_pool.tile([K, 1], F32, tag="rnv")
        nc.scalar.activation(rnv[:], lnav[:], func=AF.Exp, scale=-0.5)
        o_t = ep_pool.tile([K, C], F32, tag="o")
        # v * (1/sqrt(ssqv)) * 0.125  (global L2 norm is exactly sqrt(K)=8)
        nc.vector.tensor_scalar(o_t[:], v[:], rnv[:], 0.125,
                                op0=ALU.mult, op1=ALU.mult)
        nc.sync.dma_start(out=out_ap[n], in_=o_t[:])


def kernel(x, fc_w, fc_b, centroids):
    """Full-input entry point: shards over 8 cores, returns [N, K*C] float32."""
    from concourse.bass_utils import run_bass_kernel_spmd

    x = np.ascontiguousarray(np.asarray(x, dtype=np.float32))
    fc_w = np.ascontiguousarray(np.asarray(fc_w, dtype=np.float32))
    fc_b = np.ascontiguousarray(np.asarray(fc_b, dtype=np.float32)).reshape(1, K)
    centroids = np.ascontiguousarray(np.asarray(centroids, dtype=np.float32))

    nc = build_nc(N_PER_CORE, S)
    core_ids = list(range(N_CORES))
    in_maps = []
    for i in core_ids:
        shard = x[i * N_PER_CORE:(i + 1) * N_PER_CORE]
        in_maps.append({
            "x": shard,
            "fc_w": fc_w,
            "fc_b": fc_b,
            "centroids": centroids,
        })
    # Retry transient device failures (a crashed tenant can leave the cores
    # "unrecoverable" for a minute or two; they come back on their own).
    last_exc = None
    for attempt in range(4):
        try:
            res = run_bass_kernel_spmd(nc, in_maps, core_ids)
            break
        except Exception as e:  # noqa: BLE001
            last_exc = e
            if attempt == 3:
                raise
            import time as _time
            _time.sleep(45)
    outs = [res.results[i]["out"].reshape(N_PER_CORE, K * C) for i in range(N_CORES)]
    return np.concatenate(outs, axis=0)


# revision 60
# speedup vs baseline: 1.2756x; 1.1538x over previous
"""NetVLAD pooling kernel for Trainium2 (Bass/Tile), SPMD over 8 NeuronCores.

Reference computation (per sample n):
    x_hat = x / ||x||_C                      # L2 norm over channels, per position
    logits = fc_w @ x_hat + fc_b             # [K, S]
    soft = softmax_K(logits)                 # [K, S]
    a_sum = soft.sum(S)                      # [K]
    vlad = soft @ x_hat^T - a_sum[:,None] * centroids     # [K, C]
    vlad = intra_l2norm(vlad) ; flatten ; global l2norm

Kernel strategy (per core, data-parallel over N; group = 1024 positions =
8 tiles of 128):
  Per tile (PE, f16):
    mm-xT:   stationary x tile [c,128], rhs = I        -> xT PSUM   [s,128]
    mm-log:  same stationary,  rhs = fc_w^T (64 cols)  -> logits PSUM [s,64]
    mm-ssq:  stationary x^2 f16 tile, rhs = ones col   -> ssq PSUM  [s,1]
  PSUM layout per group: xT [128,1024]f32 (2 banks, 2 bufs), logits
  [128,512]f32 (1 bank, 2 bufs), ssq [128,8] (1 bank), vlad [64,132]
  (1 bank) = 8 banks.
  Per-group chain, layout B (s on partitions):
    ACT uses ONLY the natural_log_exp table set (ln/exp/copy/square) --
    zero table-set switches (sqrt is computed as exp(0.5*ln(ssq)); bacc's
    per-function greedy set choice is overridden post-finalize by
    _pin_act_tables, else Ln/Exp alternate natural_log <-> exp_and_others
    at ~1.3us per switch, every group):
      L = ln(ssq); rnorm = exp(-0.5 L)
      xt norm col <- exp(+0.5 L) written directly by ACT
    t1 = logits * rnorm (DVE 3D), t2 = t1 + biasg (DVE), E = exp(t2) (ACT)
    sume = reduce_K(E) (DVE 3D), qsc = rnorm/sume (DVE recip+mul)
    wt = E * qsc (DVE); xt cols 0:128 = x^T PSUM->SBUF f16 (ACT, small
    DVE share)
  mm2 (PE, f16): stationary = wt tile [s,64], moving = [xT|normv] (129)
    -> accum vlad[k,0:128], a_sum col 128 (= sum_s w*normv = sum soft)
  Epilogue per sample: vlad - a_sum*centroids, intra-norm, global norm
  = /8 folded (the final L2 norm is exactly sqrt(K)=8).

Emission is software-pipelined (front(g); tail(g-1); mid(g)) and the
x^T path stays on the PE: an XBAR transpose-DMA variant (xt_mode="dma")
was measured slower on HW (DMA-channel serialization) and the combined
[I|fcwT] single-matmul variant loses to the split form at group=1024
(drift-controlled interleaved A/B).
"""

import contextlib
import numpy as np

import concourse.bacc as bacc
import concourse.bass as bass
import concourse.mybir as mybir
import concourse.tile as tile

N, C, S, K = 16, 128, 16384, 64
N_CORES = 8
N_PER_CORE = N // N_CORES  # 2

F32 = mybir.dt.float32
F16 = mybir.dt.float16
AF = mybir.ActivationFunctionType
ALU = mybir.AluOpType
AX = mybir.AxisListType

TILE = 128           # positions per matmul tile

# tuning knobs (overridable via build_nc(opts=...))
DEFAULT_OPTS = dict(
    group=1024,         # positions per matmul/chain group (8 tiles)
    mm1_mode="split",   # "split": separate xT/logits matmuls (PSUM fits at
                        # group=1024); "combined": one [I|fcwT] mm per tile
    mm1_bufs=3,         # [combined] PSUM bufs (2 banks each)
    xt_mode="pe",
    xt_dve_cols=24,     # share of x^T PSUM->SBUF copy on DVE (rest ACT)
    xt_pool_cols=0,     # share on Pool (gpsimd 3D strided copy breaks
                        # walrus lowering -- keep 0)
    x2_engine="act",    # "dve" | "act" | "pool": who squares x for ssq
    x2_dve_cols=525,    # if set with x2_engine="act": split cols on DVE
    t2_engine="dve",    # "dve" | "pool": bias add
    load_groups=1,      # groups per x DMA / cast / square block
    xt_bufs=5,
    xf_bufs=4,
    ew_bufs=5,
    mm_xt_bufs=2,       # [split] PSUM bufs for x^T (2 banks each)
    mm_lg_bufs=2,       # [split] PSUM bufs for logits (1 bank each)
)


def build_nc(n_samples=N_PER_CORE, s_len=S, finalize=True, repeat=1, opts=None,
             repeat_mode="for"):
    """Build the Bass module for one core processing `n_samples` samples."""
    o = dict(DEFAULT_OPTS)
    if opts:
        o.update(opts)
    group = o["group"]
    tpg = group // TILE

    nc = bacc.Bacc("TRN2", target_bir_lowering=False, debug=False)

    x_d = nc.dram_tensor("x", [n_samples, C, s_len], F32, kind="ExternalInput")
    fcw_d = nc.dram_tensor("fc_w", [K, C], F32, kind="ExternalInput")
    fcb_d = nc.dram_tensor("fc_b", [1, K], F32, kind="ExternalInput")
    cent_d = nc.dram_tensor("centroids", [K, C], F32, kind="ExternalInput")
    out_d = nc.dram_tensor("out", [n_samples, K, C], F32, kind="ExternalOutput")

    n_groups = s_len // group

    with tile.TileContext(nc) as tc:
        with (
            tc.tile_pool(name="const", bufs=1) as const_pool,
            tc.tile_pool(name="xf", bufs=o["xf_bufs"]) as x_pool,
            tc.tile_pool(name="xh", bufs=o["xf_bufs"]) as xh_pool,
            tc.tile_pool(name="x2", bufs=2) as x2_pool,
            tc.tile_pool(name="xt", bufs=o["xt_bufs"]) as xt_pool,
            tc.tile_pool(name="ew", bufs=o["ew_bufs"]) as ew_pool,
            tc.tile_pool(name="sm", bufs=o.get("sm_bufs", 3)) as sm_pool,
            tc.tile_pool(name="ep", bufs=1) as ep_pool,
        ):
            # ---------------- constants ----------------
            ones_f32 = const_pool.tile([128, 128], F32, tag="ones_f32")
            nc.vector.memset(ones_f32[:], 1.0)
            ident_f32 = const_pool.tile([128, 128], F32, tag="ident_f32")
            nc.gpsimd.affine_select(
                ident_f32[:], ones_f32[:], pattern=[[1, 128]],
                compare_op=ALU.is_equal, fill=0.0, base=0, channel_multiplier=-1,
            )
            ident_f16 = const_pool.tile([128, 128], F16, tag="ident_f16")
            nc.vector.tensor_copy(ident_f16[:], ident_f32[:])
            ones_col_f16 = const_pool.tile([128, 1], F16, tag="ones_col")
            nc.vector.memset(ones_col_f16[:], 1.0)

            fcw_sb = const_pool.tile([K, C], F32, tag="fcw")
            nc.sync.dma_start(out=fcw_sb[:], in_=fcw_d.ap())
            fcb_sb = const_pool.tile([1, K], F32, tag="fcb")
            nc.sync.dma_start(out=fcb_sb[:], in_=fcb_d.ap())
            cent_sb = const_pool.tile([K, C], F32, tag="cent")
            nc.sync.dma_start(out=cent_sb[:], in_=cent_d.ap())

            fcwT = const_pool.tile([128, K], F16, tag="fcwT")
            biasg = const_pool.tile([128, tpg * K], F16, tag="biasg")
            combined = o.get("mm1_mode", "split") == "combined"
            if combined:
                rhs_const = const_pool.tile([128, 128 + K], F16,
                                            tag="rhs_const")
                nc.vector.tensor_copy(rhs_const[:, 0:128], ident_f32[:])
            else:
                rhs_const = None
            with tc.tile_pool(name="ipsum", bufs=1, space="PSUM") as ipsum_pool:
                fcwT_psum = ipsum_pool.tile([128, K], F32, tag="init")
                nc.tensor.transpose(fcwT_psum[:], fcw_sb[:], ident_f32[0:K, 0:K])
                nc.vector.tensor_copy(fcwT[:], fcwT_psum[:])
                if combined:
                    nc.vector.tensor_copy(rhs_const[:, 128:128 + K],
                                          fcwT_psum[:])

                # biasg[s, t*K + k] = fc_b[k]
                bias_psum = ipsum_pool.tile([128, K], F32, tag="init")
                nc.tensor.matmul(
                    bias_psum[:], lhsT=ones_f32[0:1, :], rhs=fcb_sb[:],
                    start=True, stop=True, skip_group_check=True,
                )
                bg_3d = biasg[:].rearrange("p (t x) -> p t x", t=tpg)
                nc.vector.tensor_copy(
                    bg_3d, bias_psum[:].unsqueeze(1).broadcast_to((128, tpg, K)))

            if combined:
                # [x^T | logits] interleaved at 256 stride (v1-style);
                # group must be 512 so 3 bufs of 2 banks fit PSUM
                mmxt_ctx = tc.tile_pool(name="mm1", bufs=o.get("mm1_bufs", 3),
                                        space="PSUM")
                mmlg_ctx = contextlib.nullcontext()
            else:
                mmxt_ctx = (
                    tc.tile_pool(name="mmxt", bufs=o["mm_xt_bufs"],
                                 space="PSUM")
                    if o["xt_mode"] == "pe" else contextlib.nullcontext()
                )
                mmlg_ctx = tc.tile_pool(name="mmlg", bufs=o["mm_lg_bufs"],
                                        space="PSUM")
            ssq_bufs = 2 if o["xt_mode"] == "dma" else 1
            with (
                mmxt_ctx as mmxt_pool,
                mmlg_ctx as mmlg_pool,
                tc.tile_pool(name="ssqp", bufs=ssq_bufs,
                             space="PSUM") as ssq_pool,
                tc.tile_pool(name="vladp", bufs=1, space="PSUM") as vlad_pool,
            ):
                env = dict(
                    o=o, group=group, tpg=tpg, n_groups=n_groups,
                    n_samples=n_samples,
                    x_pool=x_pool, xh_pool=xh_pool, x2_pool=x2_pool,
                    mmxt_pool=(None if o["xt_mode"] == "dma" else mmxt_pool),
                    mmlg_pool=mmlg_pool,
                    ssq_pool=ssq_pool, vlad_pool=vlad_pool,
                    xt_pool=xt_pool, ew_pool=ew_pool, sm_pool=sm_pool,
                    ep_pool=ep_pool,
                    ident_f16=ident_f16, fcwT=fcwT, biasg=biasg,
                    ones_col_f16=ones_col_f16, cent_sb=cent_sb,
                    rhs_const=rhs_const, combined=combined,
                )

                if repeat > 1 and repeat_mode == "unroll":
                    for _ in range(repeat):
                        _main_body(nc, x_d.ap(), out_d.ap(), env)
                elif repeat > 1 and repeat_mode.startswith("hybrid"):
                    k = int(repeat_mode.split(":")[1])
                    assert repeat % k == 0
                    with tc.For_i(0, repeat // k, 1):
                        for _ in range(k):
                            _main_body(nc, x_d.ap(), out_d.ap(), env)
                else:
                    loop_ctx = (tc.For_i(0, repeat, 1) if repeat > 1
                                else contextlib.nullcontext())
                    with loop_ctx:
                        _main_body(nc, x_d.ap(), out_d.ap(), env)

    if finalize:
        nc.finalize()
        _pin_act_tables(nc)
    return nc


def _pin_act_tables(nc):
    """Replace the per-function ACT table-set loads with one load of a set
    covering every activation function we use.

    bacc's insert_act_table_loads picks the FIRST act_info.json set
    containing each function, so an Ln/Exp mix alternates between
    `natural_log` and `exp_and_others` — a ~1.3us table DMA per switch,
    every group. All our functions (Ln, Exp, Copy, Square, Identity) live
    together in `natural_log_exp_and_others`, so one load up front
    suffices. Runs post-finalize: rewrites this module's own instructions
    only.
    """
    from concourse.hw_specs import get_activation_tables

    used = set()
    for b in nc.main_func.blocks:
        for inst in b.instructions:
            if isinstance(inst, mybir.InstActivation):
                used.add(inst.func)
    if not used:
        return
    tables = list(get_activation_tables(nc.m.arch).items())
    target = None
    for idx, (name, funcs) in enumerate(tables):
        if used <= funcs:
            target = idx
            break
    if target is None:  # no single covering set: leave the default placement
        return

    first_load = None
    for b in nc.main_func.blocks:
        keep = []
        for inst in b.instructions:
            if isinstance(inst, mybir.InstLoadActFuncSet):
                if first_load is None:
                    inst.act_func_set_id = target
                    first_load = inst
                continue  # drop all loads (the kept one is re-inserted below)
            keep.append(inst)
        b.instructions[:] = keep
    if first_load is not None:
        nc.main_func.blocks[0].instructions.insert(0, first_load)


def _main_body(nc, x_ap, out_ap, env):
    o = env["o"]
    group, tpg = env["group"], env["tpg"]
    n_samples, n_groups = env["n_samples"], env["n_groups"]
    x_pool = env["x_pool"]; xh_pool = env["xh_pool"]; x2_pool = env["x2_pool"]
    mmxt_pool = env["mmxt_pool"]; mmlg_pool = env["mmlg_pool"]
    ssq_pool = env["ssq_pool"]; vlad_pool = env["vlad_pool"]
    xt_pool = env["xt_pool"]; ew_pool = env["ew_pool"]; sm_pool = env["sm_pool"]
    ep_pool = env["ep_pool"]
    ident_f16 = env["ident_f16"]; fcwT = env["fcwT"]; biasg = env["biasg"]
    ones_col_f16 = env["ones_col_f16"]; cent_sb = env["cent_sb"]
    xt_dve = o["xt_dve_cols"]
    combined = env["combined"]; rhs_const = env["rhs_const"]
    lgrp = o.get("load_groups", 2)     # groups per x DMA/cast/x2 op
    lsz = lgrp * group

    def front_load(n, p):
        """Load + cast + square for a block of `lgrp` groups."""
        xf = x_pool.tile([128, lsz], F32)
        nc.sync.dma_start(out=xf[:], in_=x_ap[n][:, p * lsz:(p + 1) * lsz])
        xh = xh_pool.tile([128, lsz], F16)
        nc.gpsimd.tensor_copy(xh[:], xf[:])
        x2 = x2_pool.tile([128, lsz], F16)
        if o["x2_engine"] == "dve":
            nc.vector.tensor_mul(x2[:], xh[:], xh[:])
        elif o["x2_engine"] == "act":
            xd = o["x2_dve_cols"]
            if xd:
                nc.vector.tensor_mul(x2[:, 0:xd], xh[:, 0:xd], xh[:, 0:xd])
                nc.scalar.activation(x2[:, xd:], xh[:, xd:], func=AF.Square)
            else:
                nc.scalar.activation(x2[:], xh[:], func=AF.Square)
        else:
            nc.gpsimd.tensor_mul(x2[:], xh[:], xh[:])
        return {"xh": xh, "x2": x2}

    def front(n, g, blk):
        """Matmuls for group g (slices of the current load block)."""
        st = {}
        off = (g % lgrp) * group
        xh = blk["xh"][:, off:off + group]
        x2 = blk["x2"][:, off:off + group]

        if combined:
            mm1p = mmxt_pool.tile([128, tpg * 256], F32, tag="mm1p")
            st["mm1p"] = mm1p
        else:
            xTp = mmxt_pool.tile([128, group], F32, tag="xTp")
            st["xTp"] = xTp
            lgp = mmlg_pool.tile([128, tpg * K], F32)
            st["lgp"] = lgp
        ssqp = ssq_pool.tile([128, tpg], F32)
        for t in range(tpg):
            lhs = xh[:, t * TILE:(t + 1) * TILE]
            if combined:
                nc.tensor.matmul(
                    mm1p[:, t * 256: t * 256 + 128 + K], lhsT=lhs,
                    rhs=rhs_const[:],
                    start=True, stop=True, skip_group_check=True,
                )
            else:
                nc.tensor.matmul(
                    st["xTp"][:, t * TILE:(t + 1) * TILE], lhsT=lhs,
                    rhs=ident_f16[:],
                    start=True, stop=True, skip_group_check=True,
                )
                nc.tensor.matmul(
                    lgp[:, t * K:(t + 1) * K], lhsT=lhs, rhs=fcwT[:],
                    start=True, stop=True, skip_group_check=True,
                )
            nc.tensor.matmul(
                ssqp[:, t:t + 1],
                lhsT=x2[:, t * TILE:(t + 1) * TILE],
                rhs=ones_col_f16[:],
                start=True, stop=True, skip_group_check=True,
            )
        st["ssqp"] = ssqp
        return st

    def mid(n, g, st):
        """Norms + x^T copy + t1/t2 + E for group g."""
        ssqp = st["ssqp"]
        xt = xt_pool.tile([128, tpg * 132], F16)
        xt_3d = xt[:].rearrange("p (t x) -> p t x", t=tpg)
        st["xt"] = xt
        if combined:
            mm1_3d = st["mm1p"][:].rearrange("p (t x) -> p t x", t=tpg)
            lg_3d = mm1_3d[:, :, 128:128 + K]
            xT_3d = mm1_3d[:, :, 0:128]
        else:
            lg_3d = st["lgp"][:].rearrange("p (t x) -> p t x", t=tpg)
            xT_3d = st["xTp"][:].rearrange("p (t x) -> p t x", t=tpg)

        # norms from ssq via ln/exp (single ACT table set):
        #   rnorm = exp(-0.5 ln ssq) = 1/||x||; the norm column of xt gets
        #   normv = exp(+0.5 ln ssq) written directly by ACT
        lnv = sm_pool.tile([128, tpg], F32, tag="lnv")
        nc.scalar.activation(lnv[:], ssqp[:], func=AF.Ln)
        rnorm = sm_pool.tile([128, tpg], F32, tag="rnorm")
        nc.scalar.activation(rnorm[:], lnv[:], func=AF.Exp, scale=-0.5)
        nc.scalar.activation(xt_3d[:, :, 128:129], lnv[:].unsqueeze(-1),
                             func=AF.Exp, scale=0.5)
        rnorm_b = rnorm[:].unsqueeze(-1).broadcast_to((128, tpg, K))
        st["rnorm"] = rnorm

        # x^T PSUM->SBUF f16 copy
        xt_pool_cols = o.get("xt_pool_cols", 0)
        lo, hi = xt_dve, 128 - xt_pool_cols
        if lo > 0:
            nc.vector.tensor_copy(xt_3d[:, :, 0:lo], xT_3d[:, :, 0:lo])
        if hi > lo:
            nc.scalar.copy(xt_3d[:, :, lo:hi], xT_3d[:, :, lo:hi])
        if xt_pool_cols > 0:
            nc.gpsimd.tensor_copy(xt_3d[:, :, hi:128], xT_3d[:, :, hi:128])

        # t2 = raw*rnorm + bias ; E = exp(t2)
        t1 = ew_pool.tile([128, tpg * K], F16, tag="t1")
        t1_3d = t1[:].rearrange("p (t x) -> p t x", t=tpg)
        nc.vector.tensor_mul(t1_3d, lg_3d, rnorm_b)
        t2 = ew_pool.tile([128, tpg * K], F16, tag="t2")
        if o["t2_engine"] == "pool":
            nc.gpsimd.tensor_add(t2[:], t1[:], biasg[:])
        else:
            nc.vector.tensor_add(t2[:], t1[:], biasg[:])
        E = ew_pool.tile([128, tpg * K], F16, tag="E")
        nc.scalar.activation(E[:], t2[:], func=AF.Exp)
        st["E"] = E

    def tail(n, g, st, vlad_psum):
        """Softmax normalization + mm2 for group g."""
        E = st["E"]; xt = st["xt"]; rnorm = st["rnorm"]
        E_3d = E[:].rearrange("p (t x) -> p t x", t=tpg)

        sume = sm_pool.tile([128, tpg], F32, tag="sume")
        nc.vector.tensor_reduce(sume[:], E_3d, axis=AX.X, op=ALU.add)
        # qsc = rnorm / sumexp (walrus cannot lower a DVE divide, so
        # reciprocal + multiply; the multiply can ride on Pool)
        rsum = sm_pool.tile([128, tpg], F32, tag="rsum")
        nc.vector.reciprocal(rsum[:], sume[:])
        qsc = sm_pool.tile([128, tpg], F32, tag="qsc")
        if o.get("qsc_engine", "dve") == "pool":
            nc.gpsimd.tensor_mul(qsc[:], rsum[:], rnorm[:])
        else:
            nc.vector.tensor_mul(qsc[:], rsum[:], rnorm[:])
        wt = ew_pool.tile([128, tpg * K], F16, tag="wt")
        wt_3d = wt[:].rearrange("p (t x) -> p t x", t=tpg)
        q_b = qsc[:].unsqueeze(-1).broadcast_to((128, tpg, K))
        nc.vector.tensor_mul(wt_3d, E_3d, q_b)

        for t in range(tpg):
            first = (g == 0 and t == 0)
            last = (g == n_groups - 1 and t == tpg - 1)
            nc.tensor.matmul(
                vlad_psum[:, 0:129],
                lhsT=wt[:, t * K:(t + 1) * K],
                rhs=xt[:, t * 132: t * 132 + 129],
                start=first, stop=last, skip_group_check=True,
            )

    pipeline = o.get("pipeline", True)
    for n in range(n_samples):
        vlad_psum = vlad_pool.tile([K, 132], F32)
        prev = None
        blk = None
        for g in range(n_groups):
            if g % lgrp == 0:
                blk = front_load(n, g // lgrp)
            st = front(n, g, blk)
            if pipeline:
                if prev is not None:
                    tail(n, g - 1, prev, vlad_psum)
                mid(n, g, st)
                prev = st
            else:
                mid(n, g, st)
                tail(n, g, st, vlad_psum)
        if prev is not None:
            tail(n, n_groups - 1, prev, vlad_psum)

        # -------- epilogue for sample n --------
        acs = ep_pool.tile([K, C], F32, tag="acs")
        nc.vector.tensor_scalar_mul(acs[:], cent_sb[:], vlad_psum[:, 128:129])
        v = ep_pool.tile([K, C], F32, tag="v")
        nc.vector.tensor_sub(v[:], vlad_psum[:, 0:128], acs[:])
        v2 = ep_pool.tile([K, C], F32, tag="v2")
        nc.vector.tensor_mul(v2[:], v[:], v[:])
        ssqv = sm_pool.tile([K, 1], F32, tag="ssqv")
        nc.vector.tensor_reduce(ssqv[:], v2[:], axis=AX.X, op=ALU.add)
        lnav = sm_pool.tile([K, 1], F32, tag="lnav")
        nc.scalar.activation(lnav[:], ssqv[:], func=AF.Ln)
        rnv = sm_pool.tile([K, 1], F32, tag="rnv")
        nc.scalar.activation(rnv[:], lnav[:], func=AF.Exp, scale=-0.5)
        o_t = ep_pool.tile([K, C], F32, tag="o")
        # v * (1/sqrt(ssqv)) * 0.125  (global L2 norm is exactly sqrt(K)=8)
        nc.vector.tensor_scalar(o_t[:], v[:], rnv[:], 0.125,
                                op0=ALU.mult, op1=ALU.mult)
        nc.sync.dma_start(out=out_ap[n], in_=o_t[:])


def kernel(x, fc_w, fc_b, centroids):
    """Full-input entry point: shards over 8 cores, returns [N, K*C] float32."""
    from concourse.bass_utils import run_bass_kernel_spmd

    x = np.ascontiguousarray(np.asarray(x, dtype=np.float32))
    fc_w = np.ascontiguousarray(np.asarray(fc_w, dtype=np.float32))
    fc_b = np.ascontiguousarray(np.asarray(fc_b, dtype=np.float32)).reshape(1, K)
    centroids = np.ascontiguousarray(np.asarray(centroids, dtype=np.float32))

    nc = build_nc(N_PER_CORE, S)
    core_ids = list(range(N_CORES))
    in_maps = []
    for i in core_ids:
        shard = x[i * N_PER_CORE:(i + 1) * N_PER_CORE]
        in_maps.append({
            "x": shard,
            "fc_w": fc_w,
            "fc_b": fc_b,
            "centroids": centroids,
        })
    # Retry transient device failures (a crashed tenant can leave the cores
    # "unrecoverable" for a minute or two; they come back on their own).
    last_exc = None
    for attempt in range(4):
        try:
            res = run_bass_kernel_spmd(nc, in_maps, core_ids)
            break
        except Exception as e:  # noqa: BLE001
            last_exc = e
            if attempt == 3:
                raise
            import time as _time
            _time.sleep(45)
    outs = [res.results[i]["out"].reshape(N_PER_CORE, K * C) for i in range(N_CORES)]
    return np.concatenate(outs, axis=0)
